# revision 8
# baseline (speedup 1.0000x reference)
"""Trainium2 Bass kernel for nn_Attention_73478300500671 (retrieval_knn).

8-core SPMD: batch sharded 4 per core. Cross-batch retrieval einsum handled
by all-gathering the projected (transposed) q and k across cores and running
two symmetric matmul phases per core:
  phase1: S = Q_local . K_all^T   -> rowmax over m (free dim) -> t2v rows
  phase2: T = K_local . Q_all^T   -> rowmax over l (free dim) -> v2t columns
The (32,32) retrieve_logits are assembled on the host from the two raw
partial outputs (exact for any logit_scale).
Self-attention (scores/softmax/ctx/residual/LN) is computed on-chip per
local batch; probs.mean(axis=1) accumulated on-chip.
"""

import sys

for _p in ("/opt/trn_rl_repo",):
    if _p not in sys.path:
        sys.path.append(_p)

import numpy as np

import concourse.bass as bass
import concourse.mybir as mybir
import concourse.tile as tile
import concourse.bacc as bacc
from concourse.bass_utils import run_bass_kernel_spmd

# Problem constants (hardcoded; kernel.py must be self-contained)
B, L, D = 32, 196, 512
H, DH = 8, 64
N_CORES = 8
BL = B // N_CORES          # 4 local batches per core
LT = (128, 68)             # token tiles of L=196
NDC = D // 128             # 4 chunks of 128 along D
LN_EPS = 1e-6

F32 = mybir.dt.float32


def _build():
    nc = bacc.Bacc("TRN2", target_bir_lowering=False, debug=False,
                   num_devices=N_CORES)
    core_ids = list(range(N_CORES))

    # ---------------- kernel I/O ----------------
    qs = nc.dram_tensor("qs", [BL, L, D], F32, kind="ExternalInput").ap()
    ks = nc.dram_tensor("ks", [BL, L, D], F32, kind="ExternalInput").ap()
    vs = nc.dram_tensor("vs", [BL, L, D], F32, kind="ExternalInput").ap()
    wq = nc.dram_tensor("wq", [D, D], F32, kind="ExternalInput").ap()
    wk = nc.dram_tensor("wk", [D, D], F32, kind="ExternalInput").ap()
    wv = nc.dram_tensor("wv", [D, D], F32, kind="ExternalInput").ap()
    bq = nc.dram_tensor("bq", [D], F32, kind="ExternalInput").ap()
    bk = nc.dram_tensor("bk", [D], F32, kind="ExternalInput").ap()
    bv = nc.dram_tensor("bv", [D], F32, kind="ExternalInput").ap()
    gamma = nc.dram_tensor("gamma", [D], F32, kind="ExternalInput").ap()
    beta = nc.dram_tensor("beta", [D], F32, kind="ExternalInput").ap()

    out_ctx = nc.dram_tensor("out_ctx", [BL, L, D], F32,
                             kind="ExternalOutput").ap()
    out_pm = nc.dram_tensor("out_pm", [BL, L, L], F32,
                            kind="ExternalOutput").ap()
    out_t2v = nc.dram_tensor("out_t2v", [L, BL * B], F32,
                             kind="ExternalOutput").ap()
    out_v2t = nc.dram_tensor("out_v2t", [L, BL * B], F32,
                             kind="ExternalOutput").ap()

    ident_d = nc.inline_tensor(np.eye(128, dtype=np.float32), "ident128").ap()
    ones_d = nc.inline_tensor(np.ones((128, 128), dtype=np.float32),
                              "ones128").ap()

    with tile.TileContext(nc) as tc:
        _body(nc, tc, qs, ks, vs, wq, wk, wv, bq, bk, bv, gamma, beta,
              out_ctx, out_pm, out_t2v, out_v2t, ident_d, ones_d, core_ids)
    nc.compile()
    return nc


def _body(nc, tc, qs, ks, vs, wq, wk, wv, bq, bk, bv, gamma, beta,
          out_ctx, out_pm, out_t2v, out_v2t, ident_d, ones_d, core_ids):
    import contextlib
    est = contextlib.ExitStack()
    with est:
        persist = est.enter_context(tc.tile_pool(name="persist", bufs=1))
        sb_work = est.enter_context(tc.tile_pool(name="sb_work", bufs=3))
        dram = est.enter_context(tc.tile_pool(name="dram", bufs=1,
                                              space="DRAM"))

        # constants to SBUF
        ident = persist.tile([128, 128], F32, tag="ident")
        ones = persist.tile([128, 128], F32, tag="ones")
        nc.sync.dma_start(ident[:], ident_d[:])
        nc.sync.dma_start(ones[:], ones_d[:])

        # biases / affine params
        bq_sb = persist.tile([128, NDC], F32, tag="bq")   # [p, dc]
        bk_sb = persist.tile([128, NDC], F32, tag="bk")
        nc.sync.dma_start(bq_sb[:], bq.rearrange("(c p) -> p c", p=128))
        nc.sync.dma_start(bk_sb[:], bk.rearrange("(c p) -> p c", p=128))
        bv_sb = persist.tile([1, D], F32, tag="bv")
        gamma_sb = persist.tile([1, D], F32, tag="gamma1")
        beta_sb = persist.tile([1, D], F32, tag="beta1")
        nc.sync.dma_start(bv_sb[:], bv.rearrange("(a d) -> a d", a=1))
        nc.sync.dma_start(gamma_sb[:], gamma.rearrange("(a d) -> a d", a=1))
        nc.sync.dma_start(beta_sb[:], beta.rearrange("(a d) -> a d", a=1))

        # persistent per-core tensors
        qTp = [persist.tile([128, NDC, L], F32, tag=f"qTp{a}",
                            name=f"qTp{a}") for a in range(BL)]
        kTp = [persist.tile([128, NDC, L], F32, tag=f"kTp{a}",
                            name=f"kTp{a}") for a in range(BL)]
        vp = [persist.tile([128, 2, D], F32, tag=f"vp{a}",
                           name=f"vp{a}") for a in range(BL)]
        xq_nat = [persist.tile([128, 2, D], F32, tag=f"xqn{a}",
                               name=f"xqn{a}") for a in range(BL)]
        gamma_b = persist.tile([128, D], F32, tag="gamma_b")
        beta_b = persist.tile([128, D], F32, tag="beta_b")
        eps_sb = persist.tile([128, 1], F32, tag="eps")
        nc.vector.memset(eps_sb[:], LN_EPS)

        # ---------------- stage 1: broadcast gamma/beta, weights ---------
        with tc.tile_pool(name="pp_stage1", bufs=2,
                          space="PSUM") as pp1:
            gb_ps = pp1.tile([128, D], F32, tag="bcast")
            nc.tensor.matmul(gb_ps[:], lhsT=ones[0:1, :],
                             rhs=gamma_sb[0:1, :], start=True, stop=True)
            nc.scalar.copy(gamma_b[:], gb_ps[:])
            bb_ps = pp1.tile([128, D], F32, tag="bcast")
            nc.tensor.matmul(bb_ps[:], lhsT=ones[0:1, :],
                             rhs=beta_sb[0:1, :], start=True, stop=True)
            nc.scalar.copy(beta_b[:], bb_ps[:])

            # weight transposes: w [do, di] -> wT [di, NDC_do, 512ish]
            # wT[t][dci][:, do] layout: [128di, NDC, 128do] per di-chunk
            wTs = []
            for name, w in (("wq", wq), ("wk", wk), ("wv", wv)):
                wT = persist.tile([128, NDC, D], F32, tag=f"wT_{name}")
                wTs.append(wT)
                for dot in range(NDC):          # tile over do rows
                    wn = sb_work.tile([128, D], F32, tag="wnat")
                    nc.sync.dma_start(wn[:], w[dot * 128:(dot + 1) * 128, :])
                    for dci in range(NDC):      # block over di cols
                        pt = pp1.tile([128, 128], F32, tag="wtp")
                        nc.tensor.transpose(
                            pt[:], wn[:, dci * 128:(dci + 1) * 128],
                            ident[:])
                        nc.scalar.copy(
                            wT[:, dci, dot * 128:(dot + 1) * 128], pt[:])
            wqT, wkT, wvT = wTs

            # ---------------- stage 2: load X, transpose ----------------
            # xT[tensor][a]: [128di, NDC, L]
            xTq = [None] * BL
            xTk = [None] * BL
            xTv = [None] * BL
            for a in range(BL):
                for tname, xdram, store in (("q", qs, xTq), ("k", ks, xTk),
                                            ("v", vs, xTv)):
                    if tname == "q":
                        xn = xq_nat[a]          # keep for residual
                    else:
                        xn = sb_work.tile([128, 2, D], F32, tag="xnat")
                    nc.sync.dma_start(xn[:, 0, :], xdram[a, 0:128, :])
                    nc.sync.dma_start(xn[0:68, 1, :], xdram[a, 128:196, :])
                    xT = sb_work.tile([128, NDC, L], F32, tag=f"xT{tname}",
                                      bufs=2)
                    store[a] = xT
                    for tt, tsz in enumerate(LT):
                        for dc in range(NDC):
                            pt = pp1.tile([128, 128], F32, tag="wtp")
                            nc.tensor.transpose(
                                pt[:, 0:tsz],
                                xn[0:tsz, tt, dc * 128:(dc + 1) * 128],
                                ident[0:tsz, 0:tsz])
                            nc.scalar.copy(
                                xT[:, dc, tt * 128:tt * 128 + tsz],
                                pt[:, 0:tsz])

                # ------------ stage 3: projections for batch a ----------
                # q/k: psum [128do, L] per do-chunk; out transposed layout
                for which, xT, wT, bias, dst in (
                        ("q", xTq[a], wqT, bq_sb, qTp[a]),
                        ("k", xTk[a], wkT, bk_sb, kTp[a])):
                    for dco in range(NDC):
                        ps = pp1.tile([128, D], F32, tag="proj")
                        for dci in range(NDC):
                            nc.tensor.matmul(
                                ps[:, 0:L],
                                lhsT=wT[:, dci, dco * 128:(dco + 1) * 128],
                                rhs=xT[:, dci, :],
                                start=(dci == 0), stop=(dci == NDC - 1))
                        # copy with per-partition bias
                        nc.scalar.activation(
                            dst[:, dco, :], ps[:, 0:L],
                            mybir.ActivationFunctionType.Identity,
                            bias=bias[:, dco:dco + 1], scale=1.0)
                # v: token-major [t, do] + bias via K=1 ones matmul
                for tt, tsz in enumerate(LT):
                    ps = pp1.tile([128, D], F32, tag="proj")
                    for dci in range(NDC):
                        nc.tensor.matmul(
                            ps[0:tsz, :],
                            lhsT=xTv[a][:, dci, tt * 128:tt * 128 + tsz],
                            rhs=wvT[:, dci, :],
                            start=(dci == 0), stop=False)
                    nc.tensor.matmul(
                        ps[0:tsz, :], lhsT=ones[0:1, 0:tsz],
                        rhs=bv_sb[0:1, :], start=False, stop=True)
                    nc.scalar.copy(vp[a][0:tsz, tt, :], ps[0:tsz, :])

        # ---------------- stage 4: all-gather qT/kT --------------------
        gin = dram.tile([2, BL, NDC, 128, L], F32, tag="gin")
        gout = dram.tile([N_CORES, 2, BL, NDC, 128, L], F32, tag="gout",
                         addr_space="Shared")
        for a in range(BL):
            nc.sync.dma_start(gin[0, a].rearrange("d p t -> p d t"),
                              qTp[a][:])
            nc.sync.dma_start(gin[1, a].rearrange("d p t -> p d t"),
                              kTp[a][:])
        nc.gpsimd.collective_compute(
            "AllGather", mybir.AluOpType.bypass,
            replica_groups=[core_ids],
            ins=[gin.opt()], outs=[gout.opt()])

        # ---------------- pools for attention + retrieval ---------------
        with tc.tile_pool(name="pp_S", bufs=3, space="PSUM") as pp_S, \
             tc.tile_pool(name="pp_PT", bufs=2, space="PSUM") as pp_PT, \
             tc.tile_pool(name="pp_ctx", bufs=1, space="PSUM") as pp_ctx, \
             tc.tile_pool(name="pp_stat", bufs=1, space="PSUM") as pp_stat:

            # ---------------- stage 5: self-attention -------------------
            for a in range(BL):
                ctx_ps = pp_ctx.tile([128, 2, D], F32, tag="ctx")
                pm = persist.tile([128, 2, L], F32, tag=f"pm{a}")
                for h in range(H):
                    dc, r0 = h // 2, (h % 2) * 64
                    # scores psum [l, m] in 2 l-tiles
                    sc = pp_S.tile([128, 2, L], F32, tag="S")
                    for tt, tsz in enumerate(LT):
                        nc.tensor.matmul(
                            sc[0:tsz, tt, :],
                            lhsT=qTp[a][r0:r0 + 64, dc,
                                        tt * 128:tt * 128 + tsz],
                            rhs=kTp[a][r0:r0 + 64, dc, :],
                            start=True, stop=True)
                    # softmax (no max subtraction needed: |s/8| < ~8)
                    p_u = sb_work.tile([128, 2, L], F32, tag="p_u")
                    rs = sb_work.tile([128, 2, 1], F32, tag="rsum")
                    rc = sb_work.tile([128, 2, 1], F32, tag="recip")
                    for tt, tsz in enumerate(LT):
                        nc.scalar.activation(
                            p_u[0:tsz, tt, :], sc[0:tsz, tt, :],
                            mybir.ActivationFunctionType.Exp,
                            scale=0.125,
                            accum_out=rs[0:tsz, tt, :])
                        nc.vector.reciprocal(rc[0:tsz, tt, :],
                                             rs[0:tsz, tt, :])
                    p_n = sb_work.tile([128, 2, L], F32, tag="p_n")
                    for tt, tsz in enumerate(LT):
                        nc.vector.tensor_scalar_mul(
                            p_n[0:tsz, tt, :], p_u[0:tsz, tt, :],
                            rc[0:tsz, tt, 0:1])
                        if h == 0:
                            nc.vector.tensor_copy(pm[0:tsz, tt, :],
                                                  p_n[0:tsz, tt, :])
                        else:
                            nc.vector.tensor_add(pm[0:tsz, tt, :],
                                                 pm[0:tsz, tt, :],
                                                 p_n[0:tsz, tt, :])
                    # transpose p_n -> PT [m, l]
                    ptp = pp_PT.tile([128, 2, L], F32, tag="PT")
                    nc.tensor.transpose(ptp[:, 0, 0:128],
                                        p_n[:, 0, 0:128], ident[:])
                    nc.tensor.transpose(ptp[0:68, 1, 0:128],
                                        p_n[:, 0, 128:196], ident[:])
                    nc.tensor.transpose(ptp[:, 0, 128:196],
                                        p_n[0:68, 1, 0:128],
                                        ident[0:68, 0:68])
                    nc.tensor.transpose(ptp[0:68, 1, 128:196],
                                        p_n[0:68, 1, 128:196],
                                        ident[0:68, 0:68])
                    pts = sb_work.tile([128, 2, L], F32, tag="PTs")
                    nc.scalar.copy(pts[:, 0, :], ptp[:, 0, :])
                    nc.scalar.copy(pts[0:68, 1, :], ptp[0:68, 1, :])
                    # ctx[l, 64h:64h+64] = P_n @ v
                    for tt, tsz in enumerate(LT):
                        nc.tensor.matmul(
                            ctx_ps[0:tsz, tt, r0 + dc * 128:
                                   r0 + dc * 128 + 64],
                            lhsT=pts[:, 0, tt * 128:tt * 128 + tsz],
                            rhs=vp[a][:, 0, r0 + dc * 128:
                                      r0 + dc * 128 + 64],
                            start=True, stop=False)
                        nc.tensor.matmul(
                            ctx_ps[0:tsz, tt, r0 + dc * 128:
                                   r0 + dc * 128 + 64],
                            lhsT=pts[0:68, 1, tt * 128:tt * 128 + tsz],
                            rhs=vp[a][0:68, 1, r0 + dc * 128:
                                      r0 + dc * 128 + 64],
                            start=False, stop=True)
                # probs_mean out: pm/8 -> DMA
                pmo = sb_work.tile([128, 2, L], F32, tag="pmo")
                for tt, tsz in enumerate(LT):
                    nc.scalar.mul(pmo[0:tsz, tt, :], pm[0:tsz, tt, :],
                                  0.125)
                nc.sync.dma_start(out_pm[a, 0:128, :], pmo[:, 0, :])
                nc.sync.dma_start(out_pm[a, 128:196, :], pmo[0:68, 1, :])

                # residual + layernorm per l-tile
                for tt, tsz in enumerate(LT):
                    x = sb_work.tile([128, D], F32, tag="ln_x")
                    nc.vector.tensor_add(x[0:tsz, :], ctx_ps[0:tsz, tt, :],
                                         xq_nat[a][0:tsz, tt, :])
                    s1 = sb_work.tile([128, 1], F32, tag="ln_s1")
                    nc.vector.reduce_sum(s1[0:tsz, :], x[0:tsz, :],
                                         axis=mybir.AxisListType.X)
                    negmu = sb_work.tile([128, 1], F32, tag="ln_negmu")
                    nc.vector.tensor_scalar_mul(negmu[0:tsz, :],
                                                s1[0:tsz, :], -1.0 / D)
                    xc = sb_work.tile([128, D], F32, tag="ln_xc")
                    sq = sb_work.tile([128, D], F32, tag="ln_sq")
                    ssq = sb_work.tile([128, 1], F32, tag="ln_ssq")
                    nc.scalar.activation(
                        xc[0:tsz, :], x[0:tsz, :],
                        mybir.ActivationFunctionType.Identity,
                        bias=negmu[0:tsz, 0:1], scale=1.0)
                    nc.scalar.activation(
                        sq[0:tsz, :], xc[0:tsz, :],
                        mybir.ActivationFunctionType.Square,
                        accum_out=ssq[0:tsz, :])
                    std = sb_work.tile([128, 1], F32, tag="ln_std")
                    nc.scalar.activation(
                        std[0:tsz, :], ssq[0:tsz, :],
                        mybir.ActivationFunctionType.Sqrt,
                        bias=eps_sb[0:tsz, 0:1], scale=1.0 / D)
                    rstd = sb_work.tile([128, 1], F32, tag="ln_rstd")
                    nc.vector.reciprocal(rstd[0:tsz, :], std[0:tsz, :])
                    xo = sb_work.tile([128, D], F32, tag="ln_xo")
                    nc.vector.tensor_scalar_mul(xo[0:tsz, :], xc[0:tsz, :],
                                                rstd[0:tsz, 0:1])
                    nc.vector.tensor_mul(xo[0:tsz, :], xo[0:tsz, :],
                                         gamma_b[0:tsz, :])
                    nc.vector.tensor_add(xo[0:tsz, :], xo[0:tsz, :],
                                         beta_b[0:tsz, :])
                    nc.sync.dma_start(
                        out_ctx[a, tt * 128:tt * 128 + tsz, :],
                        xo[0:tsz, :])

            # ---------------- stage 6: retrieval phases -----------------
            for phase, (own, t_idx, outd) in enumerate(
                    (("q", 1, out_t2v), ("k", 0, out_v2t))):
                lhs_set = qTp if phase == 0 else kTp
                mx1 = persist.tile([128, BL, B], F32, tag=f"mx1_{phase}")
                mx2 = persist.tile([68, BL, B], F32, tag=f"mx2_{phase}")
                for b in range(B):
                    cb, ib = b // BL, b % BL
                    kb = sb_work.tile([128, NDC, L], F32, tag="stream",
                                      bufs=3)
                    nc.sync.dma_start(
                        kb[:], gout[cb, t_idx, ib].rearrange(
                            "d p t -> p d t"))
                    for a in range(BL):
                        S = pp_S.tile([128, 2, L], F32, tag="S")
                        for tt, tsz in enumerate(LT):
                            for dc in range(NDC):
                                nc.tensor.matmul(
                                    S[0:tsz, tt, :],
                                    lhsT=lhs_set[a][:, dc,
                                                    tt * 128:
                                                    tt * 128 + tsz],
                                    rhs=kb[:, dc, :],
                                    start=(dc == 0), stop=(dc == NDC - 1))
                        nc.vector.reduce_max(mx1[:, a, b:b + 1],
                                             S[:, 0, :],
                                             axis=mybir.AxisListType.X)
                        nc.vector.reduce_max(mx2[0:68, a, b:b + 1],
                                             S[0:68, 1, :],
                                             axis=mybir.AxisListType.X)
                # DMA raw max buffers; host does the partition sum
                nc.sync.dma_start(outd[0:128, :],
                                  mx1.rearrange("p a b -> p (a b)"))
                nc.sync.dma_start(outd[128:196, :],
                                  mx2.rearrange("p a b -> p (a b)"))


_NC_CACHE = None


def _get_nc():
    global _NC_CACHE
    if _NC_CACHE is None:
        _NC_CACHE = _build()
    return _NC_CACHE


def run(inputs, trace=False):
    """Run the SPMD kernel on full inputs; returns (res, outputs_tuple)."""
    nc = _get_nc()
    f = lambda x: np.ascontiguousarray(np.asarray(x, dtype=np.float32))
    q, k, v = f(inputs["query_states"]), f(inputs["key_states"]), \
        f(inputs["value_states"])
    common = dict(
        wq=f(inputs["Wq"]), wk=f(inputs["Wk"]), wv=f(inputs["Wv"]),
        bq=f(inputs["bq"]), bk=f(inputs["bk"]), bv=f(inputs["bv"]),
        gamma=f(inputs["ln_gamma"]), beta=f(inputs["ln_beta"]))
    in_maps = []
    for c in range(N_CORES):
        sl = slice(c * BL, (c + 1) * BL)
        in_maps.append(dict(qs=q[sl], ks=k[sl], vs=v[sl], **common))
    res = run_bass_kernel_spmd(nc, in_maps, list(range(N_CORES)),
                               trace=trace)
    ctx = np.concatenate([res.results[c]["out_ctx"]
                          for c in range(N_CORES)], axis=0)
    pm = np.concatenate([res.results[c]["out_pm"]
                         for c in range(N_CORES)], axis=0)
    t2v = np.concatenate(
        [res.results[c]["out_t2v"].sum(axis=0, dtype=np.float64)
         .reshape(BL, B) for c in range(N_CORES)], axis=0)       # [a, b]
    v2t_cols = np.concatenate(
        [res.results[c]["out_v2t"].sum(axis=0, dtype=np.float64)
         .reshape(BL, B) for c in range(N_CORES)], axis=0)       # [b, a]
    ls = float(np.asarray(inputs["logit_scale"]))
    logits = np.exp(ls) * (t2v + v2t_cols.T) / (2.0 * L)
    return res, (ctx.astype(np.float32), logits.astype(np.float32),
                 pm.astype(np.float32))


def kernel(**inputs):
    _, out = run(inputs, trace=False)
    return out


# revision 9
# speedup vs baseline: 2.9395x; 2.9395x over previous
"""Trainium2 Bass kernel for nn_Attention_73478300500671 (retrieval_knn).

8-core SPMD: batch sharded 4 per core. Cross-batch retrieval einsum handled
by all-gathering the projected (transposed, bf16) q and k across cores and
running two symmetric matmul phases per core:
  phase1: S = Q_local . K_all^T   -> rowmax over m (free dim) -> t2v rows
  phase2: T = K_local . Q_all^T   -> rowmax over l (free dim) -> v2t columns
Raw per-token rowmax buffers are DMA'd out; the host does the token sums
and assembles the (32,32) retrieve_logits (exact for any logit_scale).
Self-attention (scores/softmax/ctx/residual/LN) is computed on-chip per
local batch in bf16 (f32 accumulation); probs.mean(axis=1) on-chip.
"""

import sys

for _p in ("/opt/trn_rl_repo",):
    if _p not in sys.path:
        sys.path.append(_p)

import numpy as np
import ml_dtypes

import concourse.bass as bass
import concourse.mybir as mybir
import concourse.tile as tile
import concourse.bacc as bacc
from concourse.bass_utils import run_bass_kernel_spmd

# Problem constants (hardcoded; kernel.py must be self-contained)
B, L, D = 32, 196, 512
H, DH = 8, 64
N_CORES = 8
BL = B // N_CORES          # 4 local batches per core
TOK = BL * L               # 784 packed local tokens
LT = (128, 68)             # token tiles of L=196 (per-batch, attention path)
# packed token M-tiles for retrieval: 6 x 128 + 16
MT = [(i * 128, min(128, TOK - i * 128)) for i in range((TOK + 127) // 128)]
NDC = D // 128             # 4 chunks of 128 along D
LN_EPS = 1e-6

F32 = mybir.dt.float32
BF16 = mybir.dt.bfloat16
BF16_NP = ml_dtypes.bfloat16


def _build():
    nc = bacc.Bacc("TRN2", target_bir_lowering=False, debug=False,
                   num_devices=N_CORES)
    core_ids = list(range(N_CORES))

    # ---------------- kernel I/O ----------------
    qs = nc.dram_tensor("qs", [BL, L, D], F32, kind="ExternalInput").ap()
    ks = nc.dram_tensor("ks", [BL, L, D], F32, kind="ExternalInput").ap()
    vs = nc.dram_tensor("vs", [BL, L, D], F32, kind="ExternalInput").ap()
    wq = nc.dram_tensor("wq", [D, D], F32, kind="ExternalInput").ap()
    wk = nc.dram_tensor("wk", [D, D], F32, kind="ExternalInput").ap()
    wv = nc.dram_tensor("wv", [D, D], F32, kind="ExternalInput").ap()
    bq = nc.dram_tensor("bq", [D], F32, kind="ExternalInput").ap()
    bk = nc.dram_tensor("bk", [D], F32, kind="ExternalInput").ap()
    bv = nc.dram_tensor("bv", [D], F32, kind="ExternalInput").ap()
    gamma = nc.dram_tensor("gamma", [D], F32, kind="ExternalInput").ap()
    beta = nc.dram_tensor("beta", [D], F32, kind="ExternalInput").ap()

    out_ctx = nc.dram_tensor("out_ctx", [BL, L, D], F32,
                             kind="ExternalOutput").ap()
    out_pm = nc.dram_tensor("out_pm", [BL, L, L], F32,
                            kind="ExternalOutput").ap()
    # raw rowmax buffers: [128 token-rows, n_mtiles, B]
    out_t2v = nc.dram_tensor("out_t2v", [128, len(MT), B], F32,
                             kind="ExternalOutput").ap()
    out_v2t = nc.dram_tensor("out_v2t", [128, len(MT), B], F32,
                             kind="ExternalOutput").ap()

    ident_d = nc.inline_tensor(np.eye(128, dtype=np.float32), "identf").ap()
    identb_d = nc.inline_tensor(np.eye(128, dtype=BF16_NP), "identb").ap()
    onesb_d = nc.inline_tensor(np.ones((1, 128), dtype=BF16_NP),
                               "onesb").ap()
    ones_d = nc.inline_tensor(np.ones((128, 128), dtype=np.float32),
                              "ones128").ap()

    with tile.TileContext(nc) as tc:
        _body(nc, tc, qs, ks, vs, wq, wk, wv, bq, bk, bv, gamma, beta,
              out_ctx, out_pm, out_t2v, out_v2t,
              ident_d, identb_d, onesb_d, ones_d, core_ids)
    nc.compile()
    return nc


def _body(nc, tc, qs, ks, vs, wq, wk, wv, bq, bk, bv, gamma, beta,
          out_ctx, out_pm, out_t2v, out_v2t,
          ident_d, identb_d, onesb_d, ones_d, core_ids):
    import contextlib
    est = contextlib.ExitStack()
    with est:
        persist = est.enter_context(tc.tile_pool(name="persist", bufs=1))
        sb_work = est.enter_context(tc.tile_pool(name="sb_work", bufs=3))
        dram = est.enter_context(tc.tile_pool(name="dram", bufs=1,
                                              space="DRAM"))

        # constants to SBUF
        ident = persist.tile([128, 128], F32, tag="ident")
        identb = persist.tile([128, 128], BF16, tag="identb")
        onesb = persist.tile([1, 128], BF16, tag="onesb")
        ones = persist.tile([128, 128], F32, tag="ones")
        nc.sync.dma_start(ident[:], ident_d[:])
        nc.sync.dma_start(identb[:], identb_d[:])
        nc.sync.dma_start(onesb[:], onesb_d[:])
        nc.sync.dma_start(ones[:], ones_d[:])

        # biases / affine params
        bq_sb = persist.tile([128, NDC], F32, tag="bq")   # [p, dc]
        bk_sb = persist.tile([128, NDC], F32, tag="bk")
        nc.sync.dma_start(bq_sb[:], bq.rearrange("(c p) -> p c", p=128))
        nc.sync.dma_start(bk_sb[:], bk.rearrange("(c p) -> p c", p=128))
        bv_f32 = persist.tile([1, D], F32, tag="bvf")
        bv_sb = persist.tile([1, D], BF16, tag="bv")
        gamma_sb = persist.tile([1, D], F32, tag="gamma1")
        beta_sb = persist.tile([1, D], F32, tag="beta1")
        nc.sync.dma_start(bv_f32[:], bv.rearrange("(a d) -> a d", a=1))
        nc.vector.tensor_copy(bv_sb[:], bv_f32[:])
        nc.sync.dma_start(gamma_sb[:], gamma.rearrange("(a d) -> a d", a=1))
        nc.sync.dma_start(beta_sb[:], beta.rearrange("(a d) -> a d", a=1))

        # persistent per-core tensors (packed token layouts, bf16)
        qTp = persist.tile([128, NDC, TOK], BF16, tag="qTp")
        kTp = persist.tile([128, NDC, TOK], BF16, tag="kTp")
        vp = [persist.tile([128, 2, D], BF16, tag=f"vp{a}",
                           name=f"vp{a}") for a in range(BL)]
        xq_nat = [persist.tile([128, 2, D], F32, tag=f"xqn{a}",
                               name=f"xqn{a}") for a in range(BL)]
        gamma_b = persist.tile([128, D], F32, tag="gamma_b")
        beta_b = persist.tile([128, D], F32, tag="beta_b")
        eps_sb = persist.tile([128, 1], F32, tag="eps")
        nc.vector.memset(eps_sb[:], LN_EPS)

        # ---------------- stage 1: broadcast gamma/beta, weights ---------
        with tc.tile_pool(name="pp_stage1", bufs=2,
                          space="PSUM") as pp1:
            gb_ps = pp1.tile([128, D], F32, tag="bcast")
            nc.tensor.matmul(gb_ps[:], lhsT=ones[0:1, :],
                             rhs=gamma_sb[0:1, :], start=True, stop=True)
            nc.scalar.copy(gamma_b[:], gb_ps[:])
            bb_ps = pp1.tile([128, D], F32, tag="bcast")
            nc.tensor.matmul(bb_ps[:], lhsT=ones[0:1, :],
                             rhs=beta_sb[0:1, :], start=True, stop=True)
            nc.scalar.copy(beta_b[:], bb_ps[:])

            # weight transposes: w [do, di] -> wT bf16 [128di, NDC_dc, D_do]
            wTs = []
            for name, w in (("wq", wq), ("wk", wk), ("wv", wv)):
                wT = persist.tile([128, NDC, D], BF16, tag=f"wT_{name}",
                                  name=f"wT_{name}")
                wTs.append(wT)
                for dot in range(NDC):          # tile over do rows
                    wn = sb_work.tile([128, D], F32, tag="wnat")
                    nc.sync.dma_start(wn[:], w[dot * 128:(dot + 1) * 128, :])
                    for dci in range(NDC):      # block over di cols
                        pt = pp1.tile([128, 128], F32, tag="wtp")
                        nc.tensor.transpose(
                            pt[:], wn[:, dci * 128:(dci + 1) * 128],
                            ident[:])
                        nc.scalar.copy(
                            wT[:, dci, dot * 128:(dot + 1) * 128], pt[:])
            wqT, wkT, wvT = wTs

            # ---------------- stage 2+3: per batch load/transpose/proj ---
            for a in range(BL):
                xTs = {}
                for tname, xdram in (("q", qs), ("k", ks), ("v", vs)):
                    if tname == "q":
                        xn = xq_nat[a]          # keep f32 for residual
                    else:
                        xn = sb_work.tile([128, 2, D], F32, tag="xnat")
                    nc.sync.dma_start(xn[:, 0, :], xdram[a, 0:128, :])
                    nc.sync.dma_start(xn[0:68, 1, :], xdram[a, 128:196, :])
                    xT = sb_work.tile([128, NDC, L], BF16, tag="xT",
                                      bufs=4)
                    xTs[tname] = xT
                    for tt, tsz in enumerate(LT):
                        for dc in range(NDC):
                            pt = pp1.tile([128, 128], F32, tag="wtp")
                            nc.tensor.transpose(
                                pt[:, 0:tsz],
                                xn[0:tsz, tt, dc * 128:(dc + 1) * 128],
                                ident[0:tsz, 0:tsz])
                            nc.scalar.copy(
                                xT[:, dc, tt * 128:tt * 128 + tsz],
                                pt[:, 0:tsz])

                # q/k projections -> packed transposed bf16 [do, tokens]
                for which, xT, wT, bias, dst in (
                        ("q", xTs["q"], wqT, bq_sb, qTp),
                        ("k", xTs["k"], wkT, bk_sb, kTp)):
                    for dco in range(NDC):
                        ps = pp1.tile([128, D], F32, tag="proj")
                        for dci in range(NDC):
                            nc.tensor.matmul(
                                ps[:, 0:L],
                                lhsT=wT[:, dci, dco * 128:(dco + 1) * 128],
                                rhs=xT[:, dci, :],
                                start=(dci == 0), stop=(dci == NDC - 1))
                        nc.scalar.activation(
                            dst[:, dco, a * L:(a + 1) * L], ps[:, 0:L],
                            mybir.ActivationFunctionType.Identity,
                            bias=bias[:, dco:dco + 1], scale=1.0)
                # v: token-major bf16 [t, do] + bias via K=1 ones matmul
                for tt, tsz in enumerate(LT):
                    ps = pp1.tile([128, D], F32, tag="proj")
                    for dci in range(NDC):
                        nc.tensor.matmul(
                            ps[0:tsz, :],
                            lhsT=xTs["v"][:, dci, tt * 128:tt * 128 + tsz],
                            rhs=wvT[:, dci, :],
                            start=(dci == 0), stop=False)
                    nc.tensor.matmul(
                        ps[0:tsz, :], lhsT=onesb[0:1, 0:tsz],
                        rhs=bv_sb[0:1, :], start=False, stop=True)
                    nc.scalar.copy(vp[a][0:tsz, tt, :], ps[0:tsz, :])

        # ---------------- stage 4: all-gather kT then qT (bf16) ----------
        gink = dram.tile([NDC, 128, TOK], BF16, tag="gink")
        goutk = dram.tile([N_CORES, NDC, 128, TOK], BF16, tag="goutk",
                          addr_space="Shared")
        ginq = dram.tile([NDC, 128, TOK], BF16, tag="ginq")
        goutq = dram.tile([N_CORES, NDC, 128, TOK], BF16, tag="goutq",
                          addr_space="Shared")
        nc.sync.dma_start(gink.rearrange("d p t -> p d t"), kTp[:])
        nc.sync.dma_start(ginq.rearrange("d p t -> p d t"), qTp[:])
        nc.gpsimd.collective_compute(
            "AllGather", mybir.AluOpType.bypass,
            replica_groups=[core_ids],
            ins=[gink.opt()], outs=[goutk.opt()])
        nc.gpsimd.collective_compute(
            "AllGather", mybir.AluOpType.bypass,
            replica_groups=[core_ids],
            ins=[ginq.opt()], outs=[goutq.opt()])

        # ---------------- pools for attention + retrieval ---------------
        with tc.tile_pool(name="pp_S", bufs=2, space="PSUM") as pp_S, \
             tc.tile_pool(name="pp_sc", bufs=1, space="PSUM") as pp_sc, \
             tc.tile_pool(name="pp_PT", bufs=1, space="PSUM") as pp_PT, \
             tc.tile_pool(name="pp_ctx", bufs=1, space="PSUM") as pp_ctx:

            # ---------------- stage 5: self-attention -------------------
            for a in range(BL):
                ctx_ps = pp_ctx.tile([128, 2, D], F32, tag="ctx")
                pm = persist.tile([128, 2, L], F32, tag=f"pm{a}",
                                  name=f"pm{a}")
                for h in range(H):
                    dc, r0 = h // 2, (h % 2) * 64
                    sc = pp_sc.tile([128, 2, L], F32, tag="sc")
                    for tt, tsz in enumerate(LT):
                        nc.tensor.matmul(
                            sc[0:tsz, tt, :],
                            lhsT=qTp[r0:r0 + 64, dc,
                                     a * L + tt * 128:
                                     a * L + tt * 128 + tsz],
                            rhs=kTp[r0:r0 + 64, dc, a * L:(a + 1) * L],
                            start=True, stop=True)
                    # softmax (no max subtraction needed: |s/8| < ~8)
                    p_u = sb_work.tile([128, 2, L], BF16, tag="p_u")
                    rs = sb_work.tile([128, 2, 1], F32, tag="rsum")
                    rc = sb_work.tile([128, 2, 1], F32, tag="recip")
                    for tt, tsz in enumerate(LT):
                        nc.scalar.activation(
                            p_u[0:tsz, tt, :], sc[0:tsz, tt, :],
                            mybir.ActivationFunctionType.Exp,
                            scale=0.125,
                            accum_out=rs[0:tsz, tt, :])
                        nc.vector.reciprocal(rc[0:tsz, tt, :],
                                             rs[0:tsz, tt, :])
                    p_n = sb_work.tile([128, 2, L], BF16, tag="p_n")
                    for tt, tsz in enumerate(LT):
                        nc.vector.tensor_scalar_mul(
                            p_n[0:tsz, tt, :], p_u[0:tsz, tt, :],
                            rc[0:tsz, tt, 0:1])
                        if h == 0:
                            nc.vector.tensor_copy(pm[0:tsz, tt, :],
                                                  p_n[0:tsz, tt, :])
                        else:
                            nc.vector.tensor_add(pm[0:tsz, tt, :],
                                                 pm[0:tsz, tt, :],
                                                 p_n[0:tsz, tt, :])
                    # transpose p_n -> PT [m, l] (bf16 via PE)
                    ptp = pp_PT.tile([128, 2, L], BF16, tag="PT")
                    nc.tensor.transpose(ptp[:, 0, 0:128],
                                        p_n[:, 0, 0:128], identb[:])
                    nc.tensor.transpose(ptp[0:68, 1, 0:128],
                                        p_n[:, 0, 128:196], identb[:])
                    nc.tensor.transpose(ptp[:, 0, 128:196],
                                        p_n[0:68, 1, 0:128],
                                        identb[0:68, 0:68])
                    nc.tensor.transpose(ptp[0:68, 1, 128:196],
                                        p_n[0:68, 1, 128:196],
                                        identb[0:68, 0:68])
                    pts = sb_work.tile([128, 2, L], BF16, tag="PTs")
                    nc.scalar.copy(pts[:, 0, :], ptp[:, 0, :])
                    nc.scalar.copy(pts[0:68, 1, :], ptp[0:68, 1, :])
                    # ctx[l, 64h:64h+64] = P_n @ v
                    for tt, tsz in enumerate(LT):
                        nc.tensor.matmul(
                            ctx_ps[0:tsz, tt, h * 64:h * 64 + 64],
                            lhsT=pts[:, 0, tt * 128:tt * 128 + tsz],
                            rhs=vp[a][:, 0, h * 64:h * 64 + 64],
                            start=True, stop=False)
                        nc.tensor.matmul(
                            ctx_ps[0:tsz, tt, h * 64:h * 64 + 64],
                            lhsT=pts[0:68, 1, tt * 128:tt * 128 + tsz],
                            rhs=vp[a][0:68, 1, h * 64:h * 64 + 64],
                            start=False, stop=True)
                # probs_mean out: pm/8 -> DMA
                pmo = sb_work.tile([128, 2, L], F32, tag="pmo")
                for tt, tsz in enumerate(LT):
                    nc.scalar.mul(pmo[0:tsz, tt, :], pm[0:tsz, tt, :],
                                  0.125)
                nc.sync.dma_start(out_pm[a, 0:128, :], pmo[:, 0, :])
                nc.sync.dma_start(out_pm[a, 128:196, :], pmo[0:68, 1, :])

                # residual + layernorm per l-tile
                for tt, tsz in enumerate(LT):
                    x = sb_work.tile([128, D], F32, tag="ln_x")
                    nc.vector.tensor_add(x[0:tsz, :], ctx_ps[0:tsz, tt, :],
                                         xq_nat[a][0:tsz, tt, :])
                    s1 = sb_work.tile([128, 1], F32, tag="ln_s1")
                    nc.vector.reduce_sum(s1[0:tsz, :], x[0:tsz, :],
                                         axis=mybir.AxisListType.X)
                    negmu = sb_work.tile([128, 1], F32, tag="ln_negmu")
                    nc.vector.tensor_scalar_mul(negmu[0:tsz, :],
                                                s1[0:tsz, :], -1.0 / D)
                    xc = sb_work.tile([128, D], F32, tag="ln_xc")
                    sq = sb_work.tile([128, D], F32, tag="ln_sq")
                    ssq = sb_work.tile([128, 1], F32, tag="ln_ssq")
                    nc.scalar.activation(
                        xc[0:tsz, :], x[0:tsz, :],
                        mybir.ActivationFunctionType.Identity,
                        bias=negmu[0:tsz, 0:1], scale=1.0)
                    nc.scalar.activation(
                        sq[0:tsz, :], xc[0:tsz, :],
                        mybir.ActivationFunctionType.Square,
                        accum_out=ssq[0:tsz, :])
                    std = sb_work.tile([128, 1], F32, tag="ln_std")
                    nc.scalar.activation(
                        std[0:tsz, :], ssq[0:tsz, :],
                        mybir.ActivationFunctionType.Sqrt,
                        bias=eps_sb[0:tsz, 0:1], scale=1.0 / D)
                    rstd = sb_work.tile([128, 1], F32, tag="ln_rstd")
                    nc.vector.reciprocal(rstd[0:tsz, :], std[0:tsz, :])
                    xo = sb_work.tile([128, D], F32, tag="ln_xo")
                    nc.vector.tensor_scalar_mul(xo[0:tsz, :], xc[0:tsz, :],
                                                rstd[0:tsz, 0:1])
                    nc.vector.tensor_mul(xo[0:tsz, :], xo[0:tsz, :],
                                         gamma_b[0:tsz, :])
                    nc.vector.tensor_add(xo[0:tsz, :], xo[0:tsz, :],
                                         beta_b[0:tsz, :])
                    nc.sync.dma_start(
                        out_ctx[a, tt * 128:tt * 128 + tsz, :],
                        xo[0:tsz, :])

            # ---------------- stage 6: retrieval phases -----------------
            # phase 0: lhsT = local qT (packed), stream = gathered kT
            # phase 1: lhsT = local kT (packed), stream = gathered qT
            for phase, (lhs, gsrc, outd) in enumerate(
                    ((qTp, goutk, out_t2v), (kTp, goutq, out_v2t))):
                mx = persist.tile([128, len(MT), B], F32,
                                  tag=f"mx_{phase}", name=f"mx{phase}")
                for bg in range(B // 2):        # groups of 2 batches
                    kbs = []
                    for bb in range(2):
                        b = bg * 2 + bb
                        cb, ib = b // BL, b % BL
                        kb = sb_work.tile([128, NDC, L], BF16,
                                          tag="stream", bufs=6,
                                          name=f"kb{phase}_{b}")
                        nc.sync.dma_start(
                            kb[:],
                            gsrc[cb, :, :, ib * L:(ib + 1) * L]
                            .rearrange("d p t -> p d t"))
                        kbs.append(kb)
                    for mt, (m0, msz) in enumerate(MT):
                        S = pp_S.tile([128, 2, D], F32, tag="S")
                        for bb in range(2):
                            for dc in range(NDC):
                                nc.tensor.matmul(
                                    S[0:msz, bb, 0:L],
                                    lhsT=lhs[:, dc, m0:m0 + msz],
                                    rhs=kbs[bb][:, dc, :],
                                    start=(dc == 0), stop=(dc == NDC - 1))
                        nc.vector.reduce_max(
                            mx[0:msz, mt, bg * 2:bg * 2 + 2],
                            S[0:msz, 0:2, 0:L],
                            axis=mybir.AxisListType.X)
                nc.sync.dma_start(outd.rearrange("p m b -> p (m b)"),
                                  mx.rearrange("p m b -> p (m b)"))


_NC_CACHE = None


def _get_nc():
    global _NC_CACHE
    if _NC_CACHE is None:
        _NC_CACHE = _build()
    return _NC_CACHE


def _sum_mx(raw):
    """raw [128, n_mtiles, B] packed-token rowmax buffer -> [BL, B] sums."""
    n_mt = raw.shape[1]
    flat = np.transpose(np.asarray(raw, np.float64), (1, 0, 2)) \
        .reshape(n_mt * 128, B)[:TOK]
    return flat.reshape(BL, L, B).sum(axis=1)   # [BL, B]


def run(inputs, trace=False):
    """Run the SPMD kernel on full inputs; returns (res, outputs_tuple)."""
    nc = _get_nc()
    f = lambda x: np.ascontiguousarray(np.asarray(x, dtype=np.float32))
    q, k, v = f(inputs["query_states"]), f(inputs["key_states"]), \
        f(inputs["value_states"])
    common = dict(
        wq=f(inputs["Wq"]), wk=f(inputs["Wk"]), wv=f(inputs["Wv"]),
        bq=f(inputs["bq"]), bk=f(inputs["bk"]), bv=f(inputs["bv"]),
        gamma=f(inputs["ln_gamma"]), beta=f(inputs["ln_beta"]))
    in_maps = []
    for c in range(N_CORES):
        sl = slice(c * BL, (c + 1) * BL)
        in_maps.append(dict(qs=q[sl], ks=k[sl], vs=v[sl], **common))
    res = run_bass_kernel_spmd(nc, in_maps, list(range(N_CORES)),
                               trace=trace)
    ctx = np.concatenate([res.results[c]["out_ctx"]
                          for c in range(N_CORES)], axis=0)
    pm = np.concatenate([res.results[c]["out_pm"]
                         for c in range(N_CORES)], axis=0)
    t2v = np.concatenate([_sum_mx(res.results[c]["out_t2v"])
                          for c in range(N_CORES)], axis=0)      # [a, b]
    v2t_cols = np.concatenate([_sum_mx(res.results[c]["out_v2t"])
                               for c in range(N_CORES)], axis=0)  # [b, a]
    ls = float(np.asarray(inputs["logit_scale"]))
    logits = np.exp(ls) * (t2v + v2t_cols.T) / (2.0 * L)
    return res, (ctx.astype(np.float32), logits.astype(np.float32),
                 pm.astype(np.float32))


def kernel(**inputs):
    _, out = run(inputs, trace=False)
    return out
